# revision 4
# baseline (speedup 1.0000x reference)
"""Trainium2 Bass kernel for nn_Classifier_36567351558373 (nms_detection).

Strategy (8 NeuronCores, data-parallel over the N=6000 proposal axis):
  - Each core gets a 750-proposal shard of roi_features [750,14,14,64].
  - On-device per core: 7x7/7 max-pool (14x14 -> 2x2), flatten to 256,
    h = relu(flat @ W1 + b1), logits = h @ [Wc|Wr] + [bc|br],
    softmax over the 21 class logits. Outputs cls [750,21], reg [750,4].
  - Greedy sequential NMS (argsort + pairwise-IoU suppression) is an
    inherently serial scan; it runs vectorized on the host while the
    device streams the 301MB feature tensor (the memory-bound bulk).
  - Host applies the keep mask and concatenates shards.
"""

import sys

import numpy as np

for _p in ("/opt/trn_rl_repo",):
    if _p not in sys.path:
        sys.path.insert(0, _p)

from contextlib import ExitStack

import concourse.bacc as bacc
import concourse.bass as bass
import concourse.mybir as mybir
import concourse.tile as tile
from concourse import masks
from concourse.bass_utils import run_bass_kernel_spmd

N = 6000
NCORES = 8
SHARD = N // NCORES       # 750
P = 125                   # proposals per tile (6 tiles per core, no remainder)
FEAT = 14 * 14 * 64       # 12544
NH = 1024
NOUT = 25                 # 21 class logits + 4 reg
THR = 0.7
F32 = mybir.dt.float32


def build(shard=SHARD, raw_bufs=2):
    nt = shard // P
    assert shard % P == 0
    nc = bacc.Bacc("TRN2", target_bir_lowering=False, debug=False)
    feat = nc.declare_dram_parameter("feat", [shard, FEAT], F32, isOutput=False)
    w1 = nc.declare_dram_parameter("w1", [256, NH], F32, isOutput=False)
    b1 = nc.declare_dram_parameter("b1", [1, NH], F32, isOutput=False)
    wcr = nc.declare_dram_parameter("wcr", [NH, NOUT], F32, isOutput=False)
    bcr = nc.declare_dram_parameter("bcr", [1, NOUT], F32, isOutput=False)
    cls_o = nc.declare_dram_parameter("cls", [shard, 21], F32, isOutput=True)
    reg_o = nc.declare_dram_parameter("reg", [shard, 4], F32, isOutput=True)

    with tile.TileContext(nc) as tc, ExitStack() as ctx:
        const = ctx.enter_context(tc.tile_pool(name="const", bufs=1))
        rawp = ctx.enter_context(tc.tile_pool(name="raw", bufs=raw_bufs))
        s1p = ctx.enter_context(tc.tile_pool(name="s1", bufs=2))
        flatp = ctx.enter_context(tc.tile_pool(name="flat", bufs=2))
        ftp = ctx.enter_context(tc.tile_pool(name="ft", bufs=2))
        hp = ctx.enter_context(tc.tile_pool(name="h", bufs=2))
        outp = ctx.enter_context(tc.tile_pool(name="out", bufs=3))
        statp = ctx.enter_context(tc.tile_pool(name="stat", bufs=6))
        ptrp = ctx.enter_context(tc.tile_pool(name="ptr", bufs=2, space="PSUM"))
        php = ctx.enter_context(tc.tile_pool(name="ph", bufs=2, space="PSUM"))
        plp = ctx.enter_context(tc.tile_pool(name="pl", bufs=2, space="PSUM"))

        ident = const.tile([128, 128], F32)
        masks.make_identity(nc, ident[:])
        ones = const.tile([1, 128], F32)
        nc.vector.memset(ones[:], 1.0)
        # W1 with K on partitions: two 128-row chunks side by side.
        w1t = const.tile([128, 2 * NH], F32)
        nc.sync.dma_start(out=w1t[:, 0:NH], in_=w1[0:128, :])
        nc.sync.dma_start(out=w1t[:, NH : 2 * NH], in_=w1[128:256, :])
        b1t = const.tile([1, NH], F32)
        nc.sync.dma_start(out=b1t[:], in_=b1[:])
        wcrt = const.tile([128, 8 * NOUT], F32)
        for kk in range(8):
            nc.sync.dma_start(
                out=wcrt[:, kk * NOUT : (kk + 1) * NOUT],
                in_=wcr[kk * 128 : (kk + 1) * 128, :],
            )
        bcrt = const.tile([1, NOUT], F32)
        nc.sync.dma_start(out=bcrt[:], in_=bcr[:])

        for it in range(nt):
            raw = rawp.tile([P, FEAT], F32)
            nc.sync.dma_start(out=raw[:], in_=feat[it * P : (it + 1) * P, :])

            # Max-pool stage 1: over kx (7) for each px. Input flat index is
            # y*896 + (7*px+kx)*64 + c  ->  (y px kx c).
            s1 = s1p.tile([P, 14 * 2 * 64], F32)
            rv = raw[:].rearrange("p (y px kx c) -> p px y c kx", y=14, px=2, kx=7, c=64)
            sv = s1[:].rearrange("p (y px c) -> p px y c", y=14, px=2, c=64)
            for px in range(2):
                nc.vector.tensor_reduce(
                    out=sv[:, px : px + 1],
                    in_=rv[:, px : px + 1],
                    axis=mybir.AxisListType.X,
                    op=mybir.AluOpType.max,
                )
            # Max-pool stage 2: over ky (7). s1 flat index is y*128 + px*64 + c
            # = (py ky f) with f = px*64+c; output flat index py*128+f matches
            # the reference pooled.reshape(n, -1) ordering.
            flat = flatp.tile([P, 256], F32)
            s2v = s1[:].rearrange("p (py ky f) -> p py f ky", py=2, ky=7, f=128)
            fv = flat[:].rearrange("p (py f) -> p py f", py=2, f=128)
            nc.vector.tensor_reduce(
                out=fv, in_=s2v, axis=mybir.AxisListType.X, op=mybir.AluOpType.max
            )

            # Transpose flat [P,256] -> ft [256K on partitions, P] in two
            # 128-column chunks (PE transpose via identity, PSUM -> SBUF copy).
            ft = ftp.tile([128, 256], F32)
            for kk in range(2):
                pt = ptrp.tile([128, 128], F32)
                nc.tensor.transpose(
                    pt[:, :P], flat[:, kk * 128 : (kk + 1) * 128], ident[:P, :P]
                )
                nc.scalar.copy(ft[:, kk * 128 : kk * 128 + P], pt[:, :P])

            # hT chunks: out[n128, P] = W1chunk.T-free-dim trick:
            # matmul(out, lhsT=W1[k128, n128], rhs=ftT[k128, P]).
            hps = php.tile([128, NH], F32)
            for nn in range(8):
                o = hps[:, nn * 128 : nn * 128 + P]
                nc.tensor.matmul(
                    o,
                    w1t[:, nn * 128 : nn * 128 + 128],
                    ft[:, 0:P],
                    start=True,
                    stop=False,
                )
                nc.tensor.matmul(
                    o,
                    w1t[:, NH + nn * 128 : NH + nn * 128 + 128],
                    ft[:, 128 : 128 + P],
                    start=False,
                    stop=False,
                )
                nc.tensor.matmul(
                    o,
                    b1t[:, nn * 128 : (nn + 1) * 128],
                    ones[:, :P],
                    start=False,
                    stop=True,
                )
            hsb = hp.tile([128, NH], F32)
            hv_o = hsb[:].rearrange("p (n q) -> p n q", n=8)[:, :, :P]
            hv_i = hps[:].rearrange("p (n q) -> p n q", n=8)[:, :, :P]
            nc.scalar.activation(hv_o, hv_i, mybir.ActivationFunctionType.Relu)

            # logits [P, 25] = hT.T @ [Wc|Wr] + [bc|br]
            lps = plp.tile([P, NOUT], F32)
            for kk in range(8):
                nc.tensor.matmul(
                    lps[:],
                    hsb[:, kk * 128 : kk * 128 + P],
                    wcrt[:, kk * NOUT : (kk + 1) * NOUT],
                    start=(kk == 0),
                    stop=False,
                )
            nc.tensor.matmul(lps[:], ones[:, :P], bcrt[:], start=False, stop=True)

            # Softmax over the 21 class logits.
            negm = statp.tile([P, 1], F32)
            nc.vector.tensor_reduce(
                out=negm[:],
                in_=lps[:, 0:21],
                axis=mybir.AxisListType.X,
                op=mybir.AluOpType.max,
                negate=True,
            )
            e = outp.tile([P, 21], F32)
            nc.scalar.activation(
                e[:], lps[:, 0:21], mybir.ActivationFunctionType.Exp, bias=negm[:]
            )
            ssum = statp.tile([P, 1], F32)
            nc.vector.tensor_reduce(
                out=ssum[:], in_=e[:], axis=mybir.AxisListType.X, op=mybir.AluOpType.add
            )
            rec = statp.tile([P, 1], F32)
            nc.vector.reciprocal(rec[:], ssum[:])
            clsb = outp.tile([P, 21], F32)
            nc.vector.tensor_scalar_mul(clsb[:], e[:], rec[:])
            regb = outp.tile([P, 4], F32)
            nc.scalar.copy(regb[:], lps[:, 21:25])

            nc.sync.dma_start(out=cls_o[it * P : (it + 1) * P, :], in_=clsb[:])
            nc.sync.dma_start(out=reg_o[it * P : (it + 1) * P, :], in_=regb[:])

    nc.compile()
    return nc


def nms_keep_host(boxes, scores, thr=THR):
    """Greedy sequential NMS identical to the reference math (f32)."""
    boxes = np.asarray(boxes, np.float32)
    scores = np.asarray(scores, np.float32)
    n = boxes.shape[0]
    order = np.argsort(-scores, kind="stable")
    b = boxes[order]
    x1, y1, x2, y2 = b[:, 0], b[:, 1], b[:, 2], b[:, 3]
    area = (x2 - x1) * (y2 - y1)
    keep = np.ones(n, dtype=bool)
    zero = np.float32(0.0)
    for i in range(n - 1):
        if not keep[i]:
            continue
        sl = slice(i + 1, n)
        ix1 = np.maximum(x1[i], x1[sl])
        iy1 = np.maximum(y1[i], y1[sl])
        ix2 = np.minimum(x2[i], x2[sl])
        iy2 = np.minimum(y2[i], y2[sl])
        inter = np.maximum(zero, ix2 - ix1) * np.maximum(zero, iy2 - iy1)
        iou = inter / (area[i] + area[sl] - inter)
        keep[sl] &= iou < thr
    out = np.zeros(n, dtype=bool)
    out[order] = keep
    return out


_NC_CACHE = {}
LAST_RESULT = None


def _get_nc():
    if "nc" not in _NC_CACHE:
        _NC_CACHE["nc"] = build()
    return _NC_CACHE["nc"]


def kernel(**inputs):
    boxes = np.ascontiguousarray(np.asarray(inputs["boxes"], np.float32))
    scores = np.ascontiguousarray(np.asarray(inputs["scores"], np.float32))
    feat = np.ascontiguousarray(
        np.asarray(inputs["roi_features"], np.float32).reshape(N, FEAT)
    )
    w1 = np.ascontiguousarray(np.asarray(inputs["W1"], np.float32))
    b1 = np.ascontiguousarray(np.asarray(inputs["b1"], np.float32).reshape(1, NH))
    wcr = np.ascontiguousarray(
        np.concatenate(
            [np.asarray(inputs["Wc"], np.float32), np.asarray(inputs["Wr"], np.float32)],
            axis=1,
        )
    )
    bcr = np.ascontiguousarray(
        np.concatenate(
            [np.asarray(inputs["bc"], np.float32), np.asarray(inputs["br"], np.float32)]
        ).reshape(1, NOUT)
    )

    nc = _get_nc()
    in_maps = [
        dict(
            feat=feat[i * SHARD : (i + 1) * SHARD],
            w1=w1,
            b1=b1,
            wcr=wcr,
            bcr=bcr,
        )
        for i in range(NCORES)
    ]
    import os

    trace = bool(int(os.environ.get("KERNEL_TRACE", "0")))
    res = run_bass_kernel_spmd(
        nc, in_maps, core_ids=list(range(NCORES)), trace=trace
    )
    global LAST_RESULT
    LAST_RESULT = res
    cls = np.concatenate([res.results[i]["cls"] for i in range(NCORES)], axis=0)
    reg = np.concatenate([res.results[i]["reg"] for i in range(NCORES)], axis=0)

    keep = nms_keep_host(boxes, scores, THR)
    kf = keep.astype(np.float32)[:, None]
    return cls * kf, reg * kf, keep


# revision 12
# speedup vs baseline: 1.1234x; 1.1234x over previous
"""Trainium2 Bass kernel for nn_Classifier_36567351558373 (nms_detection).

Strategy (8 NeuronCores, data-parallel over the N=6000 proposal axis):
  - Each core gets a 750-proposal shard of roi_features [750,14,14,64].
  - On-device per core: 7x7/7 max-pool (14x14 -> 2x2), flatten to 256,
    h = relu(flat @ W1 + b1), logits = h @ [Wc|Wr] + [bc|br],
    softmax over the 21 class logits. Outputs cls [750,21], reg [750,4].
  - Greedy sequential NMS (argsort + pairwise-IoU suppression) is an
    inherently serial scan; it runs vectorized on the host while the
    device streams the 301MB feature tensor (the memory-bound bulk).
  - Host applies the keep mask and concatenates shards.
"""

import sys

import numpy as np

for _p in ("/opt/trn_rl_repo",):
    if _p not in sys.path:
        sys.path.insert(0, _p)

from contextlib import ExitStack

import concourse.bacc as bacc
import concourse.bass as bass
import concourse.mybir as mybir
import concourse.tile as tile
from concourse import masks
from concourse.bass_utils import run_bass_kernel_spmd

N = 6000
NCORES = 8
SHARD = N // NCORES       # 750
P = 125                   # proposals per tile (6 tiles per core, no remainder)
FEAT = 14 * 14 * 64       # 12544
NH = 1024
NOUT = 25                 # 21 class logits + 4 reg
THR = 0.7
F32 = mybir.dt.float32


def build(shard=SHARD, raw_bufs=2):
    nt = shard // P
    assert shard % P == 0
    nc = bacc.Bacc("TRN2", target_bir_lowering=False, debug=False)
    feat = nc.declare_dram_parameter("feat", [shard, FEAT], F32, isOutput=False)
    w1 = nc.declare_dram_parameter("w1", [256, NH], F32, isOutput=False)
    b1 = nc.declare_dram_parameter("b1", [1, NH], F32, isOutput=False)
    wcr = nc.declare_dram_parameter("wcr", [NH, NOUT], F32, isOutput=False)
    bcr = nc.declare_dram_parameter("bcr", [1, NOUT], F32, isOutput=False)
    cls_o = nc.declare_dram_parameter("cls", [shard, 21], F32, isOutput=True)
    reg_o = nc.declare_dram_parameter("reg", [shard, 4], F32, isOutput=True)

    with tile.TileContext(nc) as tc, ExitStack() as ctx:
        const = ctx.enter_context(tc.tile_pool(name="const", bufs=1))
        rawp = ctx.enter_context(tc.tile_pool(name="raw", bufs=raw_bufs))
        s1p = ctx.enter_context(tc.tile_pool(name="s1", bufs=2))
        tbp = ctx.enter_context(tc.tile_pool(name="tb", bufs=2))
        flatp = ctx.enter_context(tc.tile_pool(name="flat", bufs=2))
        ftp = ctx.enter_context(tc.tile_pool(name="ft", bufs=2))
        hp = ctx.enter_context(tc.tile_pool(name="h", bufs=2))
        outp = ctx.enter_context(tc.tile_pool(name="out", bufs=3))
        statp = ctx.enter_context(tc.tile_pool(name="stat", bufs=6))
        ptrp = ctx.enter_context(tc.tile_pool(name="ptr", bufs=2, space="PSUM"))
        php = ctx.enter_context(tc.tile_pool(name="ph", bufs=2, space="PSUM"))
        plp = ctx.enter_context(tc.tile_pool(name="pl", bufs=2, space="PSUM"))

        BF16 = mybir.dt.bfloat16
        ident = const.tile([128, 128], F32)
        masks.make_identity(nc, ident[:])
        ones = const.tile([1, 128], BF16)
        nc.vector.memset(ones[:], 1.0)
        # W1 with K on partitions: two 128-row chunks side by side.
        # Loaded f32, cast once to bf16 for the TensorEngine (PSUM still f32).
        w1f = const.tile([128, 2 * NH], F32)
        nc.sync.dma_start(out=w1f[:, 0:NH], in_=w1[0:128, :])
        nc.sync.dma_start(out=w1f[:, NH : 2 * NH], in_=w1[128:256, :])
        w1t = const.tile([128, 2 * NH], BF16)
        nc.scalar.copy(w1t[:], w1f[:])
        b1t = const.tile([1, NH], BF16)
        b1f = const.tile([1, NH], F32)
        nc.sync.dma_start(out=b1f[:], in_=b1[:])
        nc.scalar.copy(b1t[:], b1f[:])
        wcrf = const.tile([128, 8 * NOUT], F32)
        for kk in range(8):
            nc.sync.dma_start(
                out=wcrf[:, kk * NOUT : (kk + 1) * NOUT],
                in_=wcr[kk * 128 : (kk + 1) * 128, :],
            )
        wcrt = const.tile([128, 8 * NOUT], BF16)
        nc.scalar.copy(wcrt[:], wcrf[:])
        bcrf = const.tile([1, NOUT], F32)
        nc.sync.dma_start(out=bcrf[:], in_=bcr[:])
        bcrt = const.tile([1, NOUT], BF16)
        nc.scalar.copy(bcrt[:], bcrf[:])

        for it in range(nt):
            raw = rawp.tile([P, FEAT], F32)
            nc.sync.dma_start(out=raw[:], in_=feat[it * P : (it + 1) * P, :])

            # Max-pool stage 1: over kx (7 taps). Input flat index is
            # y*896 + (7*px+kx)*64 + c -> tap views [p, y, px, c] with
            # contiguous 64-f32 innermost runs (unit stride keeps DVE fast).
            # Tap tree split across DVE (kx 0..3) and GpSimd (kx 4..6).
            s1 = s1p.tile([P, 14 * 2 * 64], F32)
            tb = tbp.tile([P, 14 * 2 * 64], F32)

            def tap(kx):
                return raw[:].rearrange(
                    "p (y px kx c) -> p kx y px c", y=14, px=2, kx=7, c=64
                )[:, kx]

            s1v = s1[:].rearrange("p (y px c) -> p y px c", y=14, px=2, c=64)
            tbv = tb[:].rearrange("p (y px c) -> p y px c", y=14, px=2, c=64)
            nc.vector.tensor_max(s1v, tap(0), tap(1))
            nc.vector.tensor_max(tbv, tap(4), tap(5))
            nc.vector.tensor_max(s1v, s1v, tap(2))
            nc.vector.tensor_max(tbv, tbv, tap(6))
            nc.vector.tensor_max(s1v, s1v, tap(3))
            nc.vector.tensor_max(s1v, s1v, tbv)
            # Max-pool stage 2: over ky (7 taps). s1 flat index is
            # y*128 + f (f = px*64+c); output flat index py*128+f matches the
            # reference pooled.reshape(n, -1) ordering.
            flat = flatp.tile([P, 256], F32)

            def tap2(ky):
                return s1[:].rearrange("p (py ky f) -> p ky py f", py=2, ky=7, f=128)[
                    :, ky
                ]

            fv = flat[:].rearrange("p (py f) -> p py f", py=2, f=128)
            nc.vector.tensor_max(fv, tap2(0), tap2(1))
            for ky in range(2, 7):
                nc.vector.tensor_max(fv, fv, tap2(ky))

            # Transpose flat [P,256] -> ft [256K on partitions, P] in two
            # 128-column chunks (PE transpose via identity, PSUM -> SBUF copy
            # that also casts to bf16 for the TensorEngine).
            ft = ftp.tile([128, 256], BF16)
            for kk in range(2):
                pt = ptrp.tile([128, 128], F32)
                nc.tensor.transpose(
                    pt[:, :P], flat[:, kk * 128 : (kk + 1) * 128], ident[:P, :P]
                )
                nc.scalar.copy(ft[:, kk * 128 : kk * 128 + P], pt[:, :P])

            # hT chunks: out[n128, P] = W1chunk.T-free-dim trick:
            # matmul(out, lhsT=W1[k128, n128], rhs=ftT[k128, P]).
            hps = php.tile([128, NH], F32)
            for nn in range(8):
                o = hps[:, nn * 128 : nn * 128 + P]
                nc.tensor.matmul(
                    o,
                    w1t[:, nn * 128 : nn * 128 + 128],
                    ft[:, 0:P],
                    start=True,
                    stop=False,
                )
                nc.tensor.matmul(
                    o,
                    w1t[:, NH + nn * 128 : NH + nn * 128 + 128],
                    ft[:, 128 : 128 + P],
                    start=False,
                    stop=False,
                )
                nc.tensor.matmul(
                    o,
                    b1t[:, nn * 128 : (nn + 1) * 128],
                    ones[:, :P],
                    start=False,
                    stop=True,
                )
            hsb = hp.tile([128, NH], BF16)
            hv_o = hsb[:].rearrange("p (n q) -> p n q", n=8)[:, :, :P]
            hv_i = hps[:].rearrange("p (n q) -> p n q", n=8)[:, :, :P]
            nc.scalar.activation(hv_o, hv_i, mybir.ActivationFunctionType.Relu)

            # logits [P, 25] = hT.T @ [Wc|Wr] + [bc|br]
            lps = plp.tile([P, NOUT], F32)
            for kk in range(8):
                nc.tensor.matmul(
                    lps[:],
                    hsb[:, kk * 128 : kk * 128 + P],
                    wcrt[:, kk * NOUT : (kk + 1) * NOUT],
                    start=(kk == 0),
                    stop=False,
                )
            nc.tensor.matmul(lps[:], ones[:, :P], bcrt[:], start=False, stop=True)

            # Softmax over the 21 class logits.
            negm = statp.tile([P, 1], F32)
            nc.vector.tensor_reduce(
                out=negm[:],
                in_=lps[:, 0:21],
                axis=mybir.AxisListType.X,
                op=mybir.AluOpType.max,
                negate=True,
            )
            e = outp.tile([P, 21], F32)
            nc.scalar.activation(
                e[:], lps[:, 0:21], mybir.ActivationFunctionType.Exp, bias=negm[:]
            )
            ssum = statp.tile([P, 1], F32)
            nc.vector.tensor_reduce(
                out=ssum[:], in_=e[:], axis=mybir.AxisListType.X, op=mybir.AluOpType.add
            )
            rec = statp.tile([P, 1], F32)
            nc.vector.reciprocal(rec[:], ssum[:])
            clsb = outp.tile([P, 21], F32)
            nc.vector.tensor_scalar_mul(clsb[:], e[:], rec[:])
            regb = outp.tile([P, 4], F32)
            nc.scalar.copy(regb[:], lps[:, 21:25])

            nc.sync.dma_start(out=cls_o[it * P : (it + 1) * P, :], in_=clsb[:])
            nc.sync.dma_start(out=reg_o[it * P : (it + 1) * P, :], in_=regb[:])

    nc.compile()
    return nc


def nms_keep_host(boxes, scores, thr=THR):
    """Greedy sequential NMS identical to the reference math (f32)."""
    boxes = np.asarray(boxes, np.float32)
    scores = np.asarray(scores, np.float32)
    n = boxes.shape[0]
    order = np.argsort(-scores, kind="stable")
    b = boxes[order]
    x1, y1, x2, y2 = b[:, 0], b[:, 1], b[:, 2], b[:, 3]
    area = (x2 - x1) * (y2 - y1)
    keep = np.ones(n, dtype=bool)
    zero = np.float32(0.0)
    for i in range(n - 1):
        if not keep[i]:
            continue
        sl = slice(i + 1, n)
        ix1 = np.maximum(x1[i], x1[sl])
        iy1 = np.maximum(y1[i], y1[sl])
        ix2 = np.minimum(x2[i], x2[sl])
        iy2 = np.minimum(y2[i], y2[sl])
        inter = np.maximum(zero, ix2 - ix1) * np.maximum(zero, iy2 - iy1)
        iou = inter / (area[i] + area[sl] - inter)
        keep[sl] &= iou < thr
    out = np.zeros(n, dtype=bool)
    out[order] = keep
    return out


_NC_CACHE = {}
LAST_RESULT = None


def _get_nc():
    if "nc" not in _NC_CACHE:
        _NC_CACHE["nc"] = build()
    return _NC_CACHE["nc"]


def kernel(**inputs):
    boxes = np.ascontiguousarray(np.asarray(inputs["boxes"], np.float32))
    scores = np.ascontiguousarray(np.asarray(inputs["scores"], np.float32))
    feat = np.ascontiguousarray(
        np.asarray(inputs["roi_features"], np.float32).reshape(N, FEAT)
    )
    w1 = np.ascontiguousarray(np.asarray(inputs["W1"], np.float32))
    b1 = np.ascontiguousarray(np.asarray(inputs["b1"], np.float32).reshape(1, NH))
    wcr = np.ascontiguousarray(
        np.concatenate(
            [np.asarray(inputs["Wc"], np.float32), np.asarray(inputs["Wr"], np.float32)],
            axis=1,
        )
    )
    bcr = np.ascontiguousarray(
        np.concatenate(
            [np.asarray(inputs["bc"], np.float32), np.asarray(inputs["br"], np.float32)]
        ).reshape(1, NOUT)
    )

    nc = _get_nc()
    in_maps = [
        dict(
            feat=feat[i * SHARD : (i + 1) * SHARD],
            w1=w1,
            b1=b1,
            wcr=wcr,
            bcr=bcr,
        )
        for i in range(NCORES)
    ]
    import os

    trace = bool(int(os.environ.get("KERNEL_TRACE", "0")))
    res = run_bass_kernel_spmd(
        nc, in_maps, core_ids=list(range(NCORES)), trace=trace
    )
    global LAST_RESULT
    LAST_RESULT = res
    cls = np.concatenate([res.results[i]["cls"] for i in range(NCORES)], axis=0)
    reg = np.concatenate([res.results[i]["reg"] for i in range(NCORES)], axis=0)

    keep = nms_keep_host(boxes, scores, THR)
    kf = keep.astype(np.float32)[:, None]
    return cls * kf, reg * kf, keep
